# revision 20
# baseline (speedup 1.0000x reference)
"""Sparse (distance-masked) attention kernel for Trainium2, 8 NeuronCores.

Module: x[1,256,16,16,16] -> qkv proj -> 8-head attention (N=4096, hd=32)
with distance<10 mask on the 16^3 grid -> out proj.

Sharding: query-parallel. Each of the 8 cores computes ALL 8 heads for a
512-query slice (mq), plus the full K/V projection (replicated) and the
final output projection for its slice. No collectives needed; host just
concatenates the 8 output slices.

Per-core layouts (SBUF):
  X    [128, 2, 4096] f32   x as [c, n], c split in 2 chunks of 128
  QT   [128, 2, 512]  f32   q^T  [32h'+d, mq]  (chunk j holds heads 4j..4j+3)
  KT   [128, 2, 4096] f32   k^T  [32h'+d, nk]
  V    [128, 32, 264] bf16  v    [nk%128, nk//128, 33h+d] with ones col at d=32
  M    [128, 32, 512] bf16  mask [nk%128, nk//128, mq]
Scores are computed transposed: S^T[nk, mq] so softmax-normalization sums
(over nk) come from the ones-column of V during attn@V, and exp'd tiles are
directly the rhs of attn@V. No max-subtraction (scores ~ N(0,1), safe).
"""

import math
import os
from contextlib import ExitStack

import numpy as np
import ml_dtypes

P = 128
C = 256
N = 4096
MQ = 512
NH = 8
HD = 32
NCH = N // P  # 32 nk chunks
NCORES = 8
SCALE = 1.0 / math.sqrt(float(HD))

_CACHE = {}


def _build_nc(variant=None):
    variant = variant or os.environ.get("KVARIANT", "f32r")
    import concourse.bass as bass
    import concourse.bacc as bacc
    import concourse.mybir as mybir
    import concourse.tile as tile

    f32 = mybir.dt.float32
    f32r = mybir.dt.float32r if "f32r" in variant else mybir.dt.float32
    bf16 = mybir.dt.bfloat16
    Exp = mybir.ActivationFunctionType.Exp

    nc = bacc.Bacc()
    WXW = N + 4 * C + 1 + MQ  # x|wqt|wkt|wvt|pjt|pb|xq concatenated along cols
    wx_p = nc.declare_dram_parameter("wx", [C, WXW], f32r, isOutput=False)
    mask_p = nc.declare_dram_parameter("mask", [N, MQ], bf16, isOutput=False)
    out_p = nc.declare_dram_parameter("out", [C, MQ], f32, isOutput=True)

    with tile.TileContext(nc) as tc, ExitStack() as es:
        sing = es.enter_context(tc.tile_pool(name="sing", bufs=1))

        Wall = sing.tile([P, 2, WXW], f32r)
        Xsb = Wall[:, :, 0:N]
        Wq = Wall[:, :, N : N + C]
        Wk = Wall[:, :, N + C : N + 2 * C]
        Wv = Wall[:, :, N + 2 * C : N + 3 * C]
        Pj = Wall[:, :, N + 3 * C : N + 4 * C]
        Pb = Wall[:, :, N + 4 * C : N + 4 * C + 1]
        Xq = Wall[:, :, N + 4 * C + 1 : WXW]
        Msb = sing.tile([P, NCH, MQ], bf16)
        KT = sing.tile([P, 2, N], f32r)
        QT = sing.tile([P, 2, MQ], f32r)
        Vsb = sing.tile([P, NCH, NH * 33], bf16)
        ofT = sing.tile([P, 2, MQ], f32r)  # normalized out_feat^T [32h'+d, mq]
        drs = sing.tile([P, NH, 4], f32)  # denoms spread [p, h, q]; m=p*4+q
        rcr = sing.tile([P, NH, 4], f32)  # reciprocals, same layout
        rbc = sing.tile([P, 2, MQ], f32)  # recip broadcast to 32 rows per head
        dnp = sing.tile([P, 4, MQ], f32)  # per-pair denom staging (rows 32/96)
        pbf = sing.tile([P, 2, 1], f32)  # bias in plain f32 for tensor_scalar
        stg = sing.tile([P, 4, MQ], f32)  # attn numerators staged out of PSUM

        # ---- input DMAs (2 total to keep sem fan-in small) ----
        nc.sync.dma_start(out=Wall, in_=wx_p.rearrange("(j p) w -> p j w", p=P))
        nc.sync.dma_start(out=Msb, in_=mask_p.rearrange("(c p) m -> p c m", p=P))
        nc.vector.tensor_copy(pbf, Pb)

        # ---- qkv projections ----
        with tc.tile_pool(name="pps", bufs=2, space="PSUM") as pps:
            for j in range(2):
                for s in range(N // MQ):
                    ps = pps.tile([P, MQ], f32, tag="ps")
                    for cj in range(2):
                        nc.tensor.matmul(
                            ps,
                            lhsT=Wk[:, cj, P * j : P * (j + 1)],
                            rhs=Xsb[:, cj, MQ * s : MQ * (s + 1)],
                            start=(cj == 0),
                            stop=(cj == 1),
                        )
                    nc.vector.tensor_copy(KT[:, j, MQ * s : MQ * (s + 1)], ps)
            for j in range(2):
                ps = pps.tile([P, MQ], f32, tag="ps")
                for cj in range(2):
                    nc.tensor.matmul(
                        ps,
                        lhsT=Wq[:, cj, P * j : P * (j + 1)],
                        rhs=Xq[:, cj, :],
                        start=(cj == 0),
                        stop=(cj == 1),
                    )
                nc.vector.tensor_copy(QT[:, j, :], ps)
            for c in range(NCH):
                ps = pps.tile([P, MQ], f32, tag="ps")
                for cj in range(2):
                    nc.tensor.matmul(
                        ps[:, 0:C],
                        lhsT=Xsb[:, cj, P * c : P * (c + 1)],
                        rhs=Wv[:, cj, :],
                        start=(cj == 0),
                        stop=(cj == 1),
                    )
                vdst = Vsb[:, c, :].rearrange("p (h e) -> p h e", h=NH)[:, :, 0:HD]
                vsrc = ps[:, 0:C].rearrange("p (h d) -> p h d", h=NH)
                nc.vector.tensor_copy(vdst, vsrc)
        ones_ap = Vsb.rearrange("p c (h e) -> p c h e", h=NH)[:, :, :, HD : HD + 1]
        nc.vector.memset(ones_ap, 1.0)

        # ---- attention: 4 pairs of heads ----
        STB = 3 if "st3" in variant else 2
        OPB = 2 if "st3" in variant else 1
        with (
            tc.tile_pool(name="stp", bufs=STB, space="PSUM") as stp,
            tc.tile_pool(name="opp", bufs=OPB, space="PSUM") as opp,
            tc.tile_pool(name="ptp", bufs=4 if "buf4" in variant else 3) as ptp,
            tc.tile_pool(name="pmp", bufs=4 if "buf4" in variant else 3) as pmp,
        ):
            ops = []
            for p4 in range(4):
                j = p4 // 2
                offA = 64 * (p4 % 2)
                offB = offA + 32
                hA = 4 * j + (offA // 32)
                hB = hA + 1
                optag = "op" if "st3" in variant else f"op{p4}"
                op = opp.tile([P, MQ], f32, tag=optag)
                ops.append(op)
                for c in range(NCH):
                    st = stp.tile([P, 2 * MQ], f32, tag="st")
                    nc.tensor.matmul(
                        st[:, 0:MQ],
                        lhsT=KT[offA : offA + HD, j, P * c : P * (c + 1)],
                        rhs=QT[offA : offA + HD, j, :],
                        start=True,
                        stop=True,
                        tile_position=(offA, 0),
                    )
                    nc.tensor.matmul(
                        st[:, MQ : 2 * MQ],
                        lhsT=KT[offB : offB + HD, j, P * c : P * (c + 1)],
                        rhs=QT[offB : offB + HD, j, :],
                        start=True,
                        stop=True,
                        tile_position=(offB, 0),
                    )
                    if "noexp" in variant:
                        continue
                    pt = ptp.tile([P, 2 * MQ], bf16, tag="pt")
                    if "exphalf" in variant:  # WRONG RESULTS: sim cost probe only
                        nc.scalar.activation(pt[:, 0:MQ], st[:, 0:MQ], Exp, scale=SCALE)
                        nc.vector.tensor_copy(pt[:, MQ:], st[:, MQ:])
                    else:
                        nc.scalar.activation(pt, st, Exp, scale=SCALE)
                    if "exponly" in variant:
                        continue
                    pm = pmp.tile([P, 2 * MQ], bf16, tag="pm")
                    if "splitmask" in variant:
                        nc.vector.tensor_mul(pm[:, 0:MQ], pt[:, 0:MQ], Msb[:, c, :])
                        nc.vector.tensor_mul(pm[:, MQ:], pt[:, MQ:], Msb[:, c, :])
                    else:
                        msl = Msb[:, c, :]
                        mrep = bass.AP(
                            tensor=msl.tensor,
                            offset=msl.offset,
                            ap=[msl.ap[0], [0, 2], msl.ap[1]],
                        )
                        nc.vector.tensor_mul(pm, pt, mrep)
                    if "noav" in variant:
                        continue
                    nc.tensor.matmul(
                        op[0:33, :],
                        lhsT=Vsb[:, c, 33 * hA : 33 * hA + 33],
                        rhs=pm[:, 0:MQ],
                        start=(c == 0),
                        stop=(c == NCH - 1),
                        tile_position=(0, 0),
                    )
                    nc.tensor.matmul(
                        op[64:97, :],
                        lhsT=Vsb[:, c, 33 * hB : 33 * hB + 33],
                        rhs=pm[:, MQ : 2 * MQ],
                        start=(c == 0),
                        stop=(c == NCH - 1),
                        tile_position=(0, 64),
                    )
                if "noav" in variant or "noexp" in variant or "exponly" in variant:
                    nc.vector.memset(dnp[32:33, p4, :], 1.0)
                    nc.vector.memset(dnp[96:97, p4, :], 1.0)
                else:
                    # stage denominator rows (psum -> sbuf, same partition)
                    nc.vector.tensor_copy(dnp[32:33, p4, :], op[32:33, :])
                    nc.vector.tensor_copy(dnp[96:97, p4, :], op[96:97, :])
                    if "st3" in variant:
                        nc.vector.tensor_copy(stg[0:32, p4, :], op[0:32, :])
                        nc.vector.tensor_copy(stg[64:96, p4, :], op[64:96, :])

            # Spread the 8 denominator rows [1,512] across partitions [128,...,4]
            # (via a DRAM bounce, one DMA each way), one cheap reciprocal, then
            # bounce back to DRAM and broadcast to 32 rows per head.
            # Head for (k, p4) is h = 2*p4 + k; m = p*4 + q.
            dpool = es.enter_context(tc.tile_pool(name="dpool", bufs=1, space="DRAM"))
            dbuf = dpool.tile([NH, MQ], f32, tag="dbuf")  # [h, m] pre-recip
            hbuf = dpool.tile([NH, MQ], f32, tag="hbuf")  # [h, m] post-recip
            for k in range(2):
                dsl = dnp[32 + 64 * k : 33 + 64 * k, :, :]
                ddst = bass.AP(
                    tensor=dbuf.tensor,
                    offset=dbuf.offset + k * MQ,
                    ap=[[2 * MQ, 4], [1, MQ]],
                )
                nc.sync.dma_start(out=ddst, in_=dsl)
            ssrc = bass.AP(
                tensor=dbuf.tensor,
                offset=dbuf.offset,
                ap=[[4, P], [MQ, NH], [1, 4]],
            )
            nc.sync.dma_start(out=drs, in_=ssrc)
            nc.vector.reciprocal(rcr, drs)
            hdst = bass.AP(
                tensor=hbuf.tensor,
                offset=hbuf.offset,
                ap=[[4, P], [MQ, NH], [1, 4]],
            )
            nc.sync.dma_start(out=hdst, in_=rcr)
            for h in range(NH):
                j = h // 4
                hh = h % 4
                hsl = hbuf[h : h + 1, :]
                bsrc = bass.AP(
                    tensor=hsl.tensor, offset=hsl.offset, ap=[[0, HD], [1, MQ]]
                )
                nc.sync.dma_start(out=rbc[32 * hh : 32 * hh + 32, j, :], in_=bsrc)
            # normalize: ofT rows = num * recip
            for p4 in range(4):
                j = p4 // 2
                offA = 64 * (p4 % 2)
                for k, off in enumerate((offA, offA + 32)):
                    h = 4 * j + off // 32
                    hh = h % 4
                    if "noav" in variant or "noexp" in variant or "exponly" in variant:
                        nc.vector.tensor_copy(
                            ofT[32 * hh : 32 * hh + 32, j, :],
                            rbc[32 * hh : 32 * hh + 32, j, :],
                        )
                    elif "st3" in variant:
                        nc.vector.tensor_mul(
                            ofT[32 * hh : 32 * hh + 32, j, :],
                            stg[64 * k : 64 * k + 32, p4, :],
                            rbc[32 * hh : 32 * hh + 32, j, :],
                        )
                    else:
                        nc.vector.tensor_mul(
                            ofT[32 * hh : 32 * hh + 32, j, :],
                            ops[p4][32 * (2 * k) : 32 * (2 * k) + 32, :],
                            rbc[32 * hh : 32 * hh + 32, j, :],
                        )

        # ---- output projection ----
        with (
            tc.tile_pool(name="yps", bufs=2, space="PSUM") as yps,
            tc.tile_pool(name="ysb", bufs=2) as ysbp,
        ):
            for oj in range(2):
                ps = yps.tile([P, MQ], f32, tag="yps")
                for cj in range(2):
                    nc.tensor.matmul(
                        ps,
                        lhsT=Pj[:, cj, P * oj : P * (oj + 1)],
                        rhs=ofT[:, cj, :],
                        start=(cj == 0),
                        stop=(cj == 1),
                    )
                ysb = ysbp.tile([P, MQ], f32, tag="ysb")
                nc.vector.tensor_scalar_add(ysb, ps, pbf[:, oj, :])
                nc.sync.dma_start(out=out_p[P * oj : P * (oj + 1), :], in_=ysb)

    nc.compile()
    return nc


def _make_mask():
    r = np.arange(16, dtype=np.float64)
    g = np.meshgrid(r, r, r, indexing="ij")
    coords = np.stack([c.reshape(-1) for c in g], axis=1)  # [N, 3]
    d2 = ((coords[:, None, :] - coords[None, :, :]) ** 2).sum(-1)
    return np.sqrt(d2) < 10.0  # [N, N] bool


def kernel(x, qkv_w, proj_w, proj_b):
    x = np.asarray(x, dtype=np.float32)
    qkv_w = np.asarray(qkv_w, dtype=np.float32)
    proj_w = np.asarray(proj_w, dtype=np.float32)
    proj_b = np.asarray(proj_b, dtype=np.float32)

    X = np.ascontiguousarray(x.reshape(C, N))
    wqt = qkv_w[0:C, :].T
    wkt = qkv_w[C : 2 * C, :].T
    wvt = qkv_w[2 * C : 3 * C, :].T
    pjt = proj_w.T
    pb = proj_b.reshape(C, 1)
    mask = _make_mask().astype(ml_dtypes.bfloat16)

    if "nc" not in _CACHE:
        _CACHE["nc"] = _build_nc()
    nc = _CACHE["nc"]

    in_maps = []
    for i in range(NCORES):
        sl = slice(MQ * i, MQ * (i + 1))
        wx = np.ascontiguousarray(
            np.concatenate([X, wqt, wkt, wvt, pjt, pb, X[:, sl]], axis=1)
        )
        in_maps.append(
            {
                "wx": wx,
                "mask": np.ascontiguousarray(mask[:, sl]),
            }
        )

    from concourse.bass_utils import run_bass_kernel_spmd

    trace = bool(int(os.environ.get("KERNEL_TRACE", "0")))
    res = run_bass_kernel_spmd(nc, in_maps, list(range(NCORES)), trace=trace)
    _CACHE["last_result"] = res
    yt = np.concatenate([res.results[i]["out"] for i in range(NCORES)], axis=1)
    return yt.reshape(1, C, 16, 16, 16).astype(np.float32)
